# revision 1
# baseline (speedup 1.0000x reference)
"""Trainium2 Bass kernel for nn_Decoder_51659866637081 (pointer-net decoder).

Math (per batch sample, reference.py):
    enc_proj = enc_output @ W1
    scan t in 0..255 with carry (h, c, ptr):
        h, c = LSTMCell(ptr, h, c)      # z = ptr@Wk + h@Wr + b, gates i,f,g,o
        scores_t = tanh(enc_proj + h@W2) @ V
        probs_t = softmax(scores_t); ptr = x[argmax scores_t]
    out: [B, T, T] stacked probs.

Sharding: data-parallel, 8 batch samples per NeuronCore x 8 cores; inside a
core the 8 samples run as two independent 4-sample chains, software-pipelined
half a step apart so one chain's serial LSTM/argmax segment overlaps the
other chain's tanh/V-dot work.

Layouts: contraction dims on partitions ([128(k), batch] style), all inputs
packed host-side into one [128, CBLOB] fp32 blob (single DMA & semaphore).
b is folded in by augmenting ptr^T with a constant 1.0 row. Argmax is done
without index math: m = rowmax(scores); ptr_c = sum((scores==m) * x_c) via
one fused scalar_tensor_tensor per coordinate; ptr^T comes from a 32x32
vector-engine block transpose. All matmuls fp32 (argmax margins down to
2.4e-5 rule out tf32-like fp32r). Scores are staged to an SBUF history and
softmax-normalized in a bulk phase after the scan (fenced so its Exp does
not thrash the ACT table against the scan's Sigmoid/Tanh set).

All four LSTM gates run in ONE sigmoid instruction: the g-gate uses
tanh(x) = 2*sigmoid(2x) - 1 with its Wr/Wk/b columns doubled host-side, and
i*g = 2*(i*sg) - i via one tensor_mul + one fused scalar_tensor_tensor.

PSUM budget: 1 bank enc_proj staging + per-chain zq pools (2 bufs each) +
per-chain score banks = 7 of 8 banks.
"""
import numpy as np

B, T, H = 64, 256, 128
NCORES = 8
BL = B // NCORES          # 8 samples per core
CH = 2                    # chains per core
BC = BL // CH             # 4 samples per chain
G4 = 4 * H
SLOT_SRC = [0, 1, 3, 2]   # z slot j <- reference gate block (i,f,o,g order)

# ---- blob column layout (fp32, [128, CBLOB]) ----
ENC0 = 0                  # encT [128, 2048] (b-major: col b*T+t)
W1C = ENC0 + BL * T
W2C = W1C + H
WRC = W2C + H             # Wr slot-reordered [128, 512]
WKC = WRC + G4            # Wk_aug rows 0:3 = [Wk0, Wk1, b] slot-reordered
VMC = WKC + G4            # V-masks [128, 2*BC*BC] (chain X tile b': col X*16+5b')
XC = VMC + 2 * BC * BC    # x rows 0:4: [4, 1024] chain-major then c-major then t
HC = XC + 2 * 2 * T       # h0T [128, 8]
CC = HC + BL              # c0T [128, 8]
PC = CC + BL              # ptr0T_aug [3, 8] = ones
CBLOB = PC + BL

_CACHE = {}


def _build(repeat=1):
    import concourse.bacc as bacc
    import concourse.tile as tile
    import concourse.mybir as mybir
    from contextlib import ExitStack

    F32 = mybir.dt.float32
    AF = mybir.ActivationFunctionType
    ALU = mybir.AluOpType
    AX = mybir.AxisListType

    nc = bacc.Bacc("TRN2", debug=False, num_devices=NCORES)
    blob = nc.dram_tensor("blob", [128, CBLOB], F32, kind="ExternalInput").ap()
    probs = nc.dram_tensor("probs", [BL, T, T], F32, kind="ExternalOutput").ap()

    with tile.TileContext(nc) as tc:
        with (
            tc.tile_pool(name="cst", bufs=1) as cst,
            tc.tile_pool(name="hist", bufs=1) as hist,
            tc.tile_pool(name="st", bufs=1) as st,
            tc.tile_pool(name="atile", bufs=2) as atile,
            tc.tile_pool(name="small", bufs=3) as small,
            tc.tile_pool(name="encp_p", bufs=1, space="PSUM") as encp_p,
            tc.tile_pool(name="zq_p0", bufs=2, space="PSUM") as zq_p0,
            tc.tile_pool(name="zq_p1", bufs=2, space="PSUM") as zq_p1,
            tc.tile_pool(name="sc_p0", bufs=1, space="PSUM") as sc_p0,
            tc.tile_pool(name="sc_p1", bufs=1, space="PSUM") as sc_p1,
        ):
            bl = cst.tile([128, CBLOB], F32)
            nc.sync.dma_start(bl[:], blob)
            encT = bl[:, ENC0:ENC0 + BL * T]
            w1 = bl[:, W1C:W1C + H]
            w2 = bl[:, W2C:W2C + H]
            wr = bl[:, WRC:WRC + G4]
            wk = bl[0:3, WKC:WKC + G4]

            encp = cst.tile([128, BL * T], F32)
            for i in range(4):
                etmp = encp_p.tile([128, 512], F32, name=f"etmp{i}", tag="etmp")
                nc.tensor.matmul(etmp[:], w1, encT[:, 512 * i:512 * (i + 1)],
                                 start=True, stop=True)
                nc.vector.tensor_copy(encp[:, 512 * i:512 * (i + 1)], etmp[:])

            scores_all = hist.tile([128, T * 16], F32)

            rep_ctx = ExitStack()
            if repeat > 1:
                rep_ctx.enter_context(tc.For_i(0, repeat, 1))

            # per-chain persistent state
            hT, cT, ptr_in, ptr_out, zq = [], [], [], [], []
            for X in range(CH):
                hT.append(st.tile([128, BC], F32, name=f"hT{X}", tag=f"hT{X}"))
                cT.append(st.tile([128, BC], F32, name=f"cT{X}", tag=f"cT{X}"))
                pi = st.tile([32, 32], F32, name=f"pi{X}", tag=f"pi{X}")
                po = st.tile([32, 32], F32, name=f"po{X}", tag=f"po{X}")
                nc.vector.memset(pi[:], 0.0)
                nc.vector.memset(pi[0:BC, 2:3], 1.0)
                ptr_in.append(pi)
                ptr_out.append(po)
                z = (zq_p0 if X == 0 else zq_p1).tile([128, 24], F32, name=f"zq{X}", tag="zq")
                for g in range(4):
                    nc.tensor.matmul(z[:, BC * g:BC * (g + 1)],
                                     wr[:, 128 * g:128 * g + 128],
                                     bl[:, HC + BC * X:HC + BC * X + BC],
                                     start=(g == 0), stop=False)
                for g in range(4):
                    nc.tensor.matmul(z[:, BC * g:BC * (g + 1)],
                                     wk[:, 128 * g:128 * g + 128],
                                     bl[0:3, PC + BC * X:PC + BC * X + BC],
                                     start=False, stop=(g == 3))
                zq.append(z)

            NG = BC * 4  # 16: z columns; q at 16:20
            znext = [None, None]
            qsb_t = [None, None]

            def lstm_phase(X, t):
                z = zq[X]
                c_prev = bl[:, CC + BC * X:CC + BC * X + BC] if t == 0 \
                    else cT[X][:]
                ifo = small.tile([128, 4 * BC], F32, name=f"ifo{X}", tag=f"ifo{X}")
                nc.scalar.activation(ifo[:], z[:, 0:4 * BC], AF.Sigmoid)
                # g-slot weights doubled on host: sg = sigmoid(2*zg),
                # i*g = i*(2*sg - 1) = 2*(i*sg) - i
                u = small.tile([128, BC], F32, name=f"u{X}", tag=f"u{X}")
                ig = small.tile([128, BC], F32, name=f"ig{X}", tag=f"ig{X}")
                fc = small.tile([128, BC], F32, name=f"fc{X}", tag=f"fc{X}")
                nc.vector.tensor_mul(u[:], ifo[:, 0:BC], ifo[:, 3 * BC:4 * BC])
                nc.vector.scalar_tensor_tensor(
                    ig[:], u[:], 2.0, ifo[:, 0:BC],
                    op0=ALU.mult, op1=ALU.subtract)
                nc.vector.tensor_mul(fc[:], ifo[:, BC:2 * BC], c_prev)
                nc.vector.tensor_add(cT[X][:], ig[:], fc[:])
                tnc = small.tile([128, BC], F32, name=f"tnc{X}", tag=f"tnc{X}")
                nc.scalar.activation(tnc[:], cT[X][:], AF.Tanh)
                nc.vector.tensor_mul(hT[X][:], ifo[:, 2 * BC:3 * BC], tnc[:])
                nc.tensor.matmul(z[:, NG:NG + BC], w2, hT[X][:],
                                 start=True, stop=True)
                qsb = small.tile([128, BC], F32, name=f"qsb{X}", tag=f"qsb{X}")
                nc.vector.tensor_copy(qsb[:], z[:, NG:NG + BC])
                qsb_t[X] = qsb
                if t < T - 1:
                    zn = (zq_p0 if X == 0 else zq_p1).tile(
                        [128, 24], F32, name=f"zqn{X}", tag="zq")
                    for g in range(4):
                        nc.tensor.matmul(zn[:, BC * g:BC * (g + 1)],
                                         wr[:, 128 * g:128 * g + 128],
                                         hT[X][:], start=(g == 0), stop=False)
                    znext[X] = zn

            sc_t = [None, None]

            def attn_phase(X, t):
                qsb = qsb_t[X]
                sc = (sc_p0 if X == 0 else sc_p1).tile(
                    [BC, T], F32, name=f"sc{X}", tag="sc")
                for b in range(BC):
                    gb = BC * X + b
                    Ab = atile.tile([128, T], F32, name=f"A{X}_{b}",
                                    tag=f"A{X}_{b}")
                    nc.scalar.activation(Ab[:],
                                         encp[:, 256 * gb:256 * (gb + 1)],
                                         AF.Tanh, bias=qsb[:, b:b + 1])
                    nc.tensor.matmul(
                        sc[:], bl[:, VMC + 16 * X + 4 * b:
                                   VMC + 16 * X + 4 * b + 4],
                        Ab[:], start=(b == 0), stop=(b == BC - 1))
                sc_t[X] = sc

            def post_phase(X, t):
                sc = sc_t[X]
                stg = small.tile([BC, T], F32, name=f"stg{X}", tag=f"stg{X}")
                nc.vector.tensor_copy(stg[:], sc[:])
                r = t % 16
                cchunk = t // 16
                nc.gpsimd.dma_start(
                    scores_all[8 * r + BC * X:8 * r + BC * X + BC,
                               256 * cchunk:256 * (cchunk + 1)],
                    stg[:])
                if t == T - 1:
                    return
                m = small.tile([BC, 1], F32, name=f"m{X}", tag=f"m{X}")
                nc.vector.reduce_max(m[:], stg[:], axis=AX.X)
                junk = small.tile([BC, T], F32, name=f"junk{X}", tag=f"junk{X}")
                for c in range(2):
                    nc.vector.scalar_tensor_tensor(
                        junk[:], stg[:], m[:],
                        bl[0:BC, XC + 512 * X + 256 * c:
                           XC + 512 * X + 256 * c + 256],
                        op0=ALU.is_equal, op1=ALU.mult,
                        accum_out=ptr_in[X][0:BC, c:c + 1])
                nc.vector.transpose(ptr_out[X][:], ptr_in[X][:])
                zn = znext[X]
                for g in range(4):
                    nc.tensor.matmul(zn[:, BC * g:BC * (g + 1)],
                                     wk[:, 128 * g:128 * g + 128],
                                     ptr_out[X][0:3, 0:BC],
                                     start=False, stop=(g == 3))
                zq[X] = zn

            # software pipeline: X1 runs half a step behind X0
            lstm_phase(0, 0)
            attn_phase(0, 0)
            lstm_phase(1, 0)
            for t in range(T):
                post_phase(0, t)
                attn_phase(1, t)
                if t + 1 < T:
                    lstm_phase(0, t + 1)
                post_phase(1, t)
                if t + 1 < T:
                    attn_phase(0, t + 1)
                    lstm_phase(1, t + 1)

            tc.no_sync_barrier()
            for cchunk in range(16):
                ss = scores_all[:, 256 * cchunk:256 * (cchunk + 1)]
                nm = small.tile([128, 1], F32, tag="nm")
                nc.vector.reduce_max(nm[:], ss, axis=AX.X, negate=True)
                ex = small.tile([128, T], F32, tag="ex")
                sm = small.tile([128, 1], F32, tag="sm")
                nc.scalar.activation(ex[:], ss, AF.Exp, bias=nm[:],
                                     accum_out=sm[:])
                rc = small.tile([128, 1], F32, tag="rc")
                nc.vector.reciprocal(rc[:], sm[:])
                pr = small.tile([128, T], F32, tag="pr")
                nc.vector.tensor_scalar_mul(pr[:], ex[:], rc[:])
                out_ap = probs[:, 16 * cchunk:16 * (cchunk + 1), :] \
                    .rearrange("b s t -> s b t")
                nc.sync.dma_start(out_ap, pr[:])
            rep_ctx.close()

    nc.compile()
    return nc


def _prep_core_blob(x, enc, h0, c0, W1, W2, V, Wk, b, Wr, core):
    lo = core * BL
    blob = np.zeros((128, CBLOB), dtype=np.float32)
    e = enc[lo:lo + BL]
    blob[:, ENC0:ENC0 + BL * T] = e.transpose(2, 0, 1).reshape(H, BL * T)
    blob[:, W1C:W1C + H] = W1
    blob[:, W2C:W2C + H] = W2
    for j, g in enumerate(SLOT_SRC):
        s = 2.0 if j == 3 else 1.0   # g-gate via sigmoid: tanh(x)=2*sig(2x)-1
        blob[:, WRC + 128 * j:WRC + 128 * (j + 1)] = s * Wr[:, 128 * g:128 * (g + 1)]
        blob[0:2, WKC + 128 * j:WKC + 128 * (j + 1)] = s * Wk[:, 128 * g:128 * (g + 1)]
        blob[2, WKC + 128 * j:WKC + 128 * (j + 1)] = s * b[128 * g:128 * (g + 1)]
    for X in range(CH):
        for s in range(BC):
            blob[:, VMC + 16 * X + 5 * s] = V
    xs = x[lo:lo + BL]
    for X in range(CH):
        for c in range(2):
            blob[0:BC, XC + 512 * X + 256 * c:XC + 512 * X + 256 * (c + 1)] = \
                xs[BC * X:BC * (X + 1), :, c]
    blob[:, HC:HC + BL] = h0[lo:lo + BL].T
    blob[:, CC:CC + BL] = c0[lo:lo + BL].T
    blob[0:3, PC:PC + BL] = 1.0
    return blob


def kernel(x, enc_output, h0, c0, W1, W2, V, Wk, Wr, b):
    from concourse.bass_utils import run_bass_kernel_spmd

    args = [np.asarray(a, dtype=np.float32)
            for a in (x, enc_output, h0, c0, W1, W2, V, Wk, b, Wr)]
    if "nc" not in _CACHE:
        _CACHE["nc"] = _build()
    nc = _CACHE["nc"]
    in_maps = [{"blob": _prep_core_blob(*args, core)} for core in range(NCORES)]
    res = run_bass_kernel_spmd(nc, in_maps, core_ids=list(range(NCORES)))
    _CACHE["last_results"] = res
    return np.concatenate([r["probs"] for r in res.results], axis=0)

